# revision 36
# baseline (speedup 1.0000x reference)
"""Trainium2 Bass kernel for the Gaussian-mixture image renderer (nn_MoE).

Math. out[a,h,w] = sum_k w_k e_k / sum_k e_k with
  e_k = exp(q), q = c00 + c10 x + c01 y + c20 x^2 + c11 x y + c02 y^2,
  x = lin[h], y = lin[w], lin = linspace(0,1,256), and c11 <= 0 always
  (c11 = -((l00+l10)^2 + l11^2)/2).
Factor e_k = u_k(x) * v_k(y) * exp(c11 x y) and Chebyshev-interpolate the
cross term in x with M_k nodes:
  exp(c x y) ~= sum_s L_s(x) exp(c x_s y)        (L_s = Lagrange basis)
so each image becomes a rank-R product (R = sum_k M_k <= 128):
  S1 = F^T G, S2 = (wF)^T G, out = S2/S1
  F[(k,s), i] = u_k(x_i) L_s(x_i),  G[(k,s), j] = v_k(y_j) exp(c11 x_s y_j).
M_k is looked up from a precomputed accuracy table (tol 3e-5); per-image
rank stays ~40-95 for this data. u,v are max-normalized so all factors
are <= O(1); the per-image scale cancels in S2/S1.

Device strategy (8 cores, data-parallel over images): core c renders
images 3c..3c+2. Per image: DMA F,G (128x256 f32); one DVE op builds
F2 = w*F; two f32r matmuls (lhsT = G j-halves, rhs = [F|F2]) produce
S1,S2 for 128 j-columns x 256 i; DVE reciprocal+multiply; DMA out.
~30 instructions/core total - no per-pixel exp, no q-build.
"""

import sys

if "/opt/trn_rl_repo" not in sys.path:
    sys.path.insert(0, "/opt/trn_rl_repo")

from contextlib import ExitStack

import numpy as np

K = 16
A = 24
H = W = 256
N_CORES = 8
IPC = 3  # images per core
RANK = 128
N_WARM = 5

# max |c11| handled by M interpolation nodes at tol 3e-5 (precomputed)
M_THRESH = [
    (1, 0.004),
    (2, 0.0217),
    (3, 0.1833),
    (4, 0.5740),
    (5, 1.2387),
    (6, 2.1383),
    (7, 3.3424),
    (8, 4.7308),
    (9, 6.3718),
    (10, 8.3716),
    (11, 10.4665),
    (12, 12.7648),
    (13, 15.1864),
    (14, 18.0673),
    (15, 19.9526),
]


# ----------------------------------------------------------------------------
# Host-side factorization
# ----------------------------------------------------------------------------

def _coefs(params):
    p = np.asarray(params, np.float64).reshape(A, 7 * K)
    mu0, mu1 = p[:, :K], p[:, K : 2 * K]
    wl = p[:, 2 * K : 3 * K]
    w = np.exp(wl - wl.max(1, keepdims=True))
    w /= w.sum(1, keepdims=True)
    raw = p[:, 3 * K : 7 * K].reshape(A, K, 2, 2)
    l00, l10, l11 = raw[:, :, 0, 0], raw[:, :, 1, 0], raw[:, :, 1, 1]
    s0 = l00 * l00 + l00 * l10
    s1 = l00 * l10 + l10 * l10 + l11 * l11
    s01 = s0 + s1
    return dict(
        c00=-0.5 * (s0 * mu0**2 + s01 * mu0 * mu1 + s1 * mu1**2),
        c10=0.5 * (2 * s0 * mu0 + s01 * mu1),
        c01=0.5 * (s01 * mu0 + 2 * s1 * mu1),
        c20=-0.5 * s0,
        c11=-0.5 * s01,
        c02=-0.5 * s1,
        w=w,
    )


def _qmax01(b, c):
    """max over t in [0,1] of b t + c t^2 (scalars)."""
    best = max(0.0, b + c)
    if abs(c) > 1e-30:
        t = -b / (2 * c)
        if 0.0 < t < 1.0:
            best = max(best, b * t + c * t * t)
    return best


def _pick_m(cabs):
    for m, thr in M_THRESH:
        if cabs <= thr:
            return m
    return min(15 + int(np.ceil((cabs - 20.0) / 2.0)), 24)


_L_CACHE = {}


def _nodes_lagrange(M):
    """Chebyshev nodes on [0,1] and Lagrange basis on the 256 grid."""
    if M in _L_CACHE:
        return _L_CACHE[M]
    x = np.linspace(0.0, 1.0, 256)
    if M == 1:
        nd = np.array([0.5])
        L = np.ones((1, 256))
    else:
        t = np.cos(np.pi * (2 * np.arange(M) + 1) / (2 * M))
        nd = 0.5 * (t + 1.0)
        wts = np.ones(M)
        for s in range(M):
            wts[s] = 1.0 / np.prod(nd[s] - np.delete(nd, s))
        d = x[None, :] - nd[:, None]
        exact = np.isclose(d, 0.0, atol=1e-13)
        d_safe = np.where(exact, 1.0, d)
        terms = wts[:, None] / d_safe
        L = terms / terms.sum(0)
        hit = exact.any(0)
        if hit.any():
            L[:, hit] = exact[:, hit].astype(float)
    _L_CACHE[M] = (nd, L)
    return nd, L


def _factor_image(C, a):
    """-> F (128,256) f32, G (128,256) f32, wrow (128,) f32."""
    x = np.linspace(0.0, 1.0, 256)
    y = x
    Mu = np.array(
        [C["c00"][a, k] + _qmax01(C["c10"][a, k], C["c20"][a, k]) for k in range(K)]
    )
    Mv = np.array([_qmax01(C["c01"][a, k], C["c02"][a, k]) for k in range(K)])
    sup = (Mu + Mv) - (Mu + Mv).max()
    Ms = [_pick_m(abs(C["c11"][a, k])) for k in range(K)]
    while sum(Ms) > RANK:
        Ms[int(np.argmax(Ms))] -= 1
    F = np.zeros((RANK, 256), np.float32)
    G = np.zeros((RANK, 256), np.float32)
    wrow = np.zeros(RANK, np.float32)
    r0 = 0
    for k in range(K):
        M = Ms[k]
        nd, L = _nodes_lagrange(M)
        qu = C["c00"][a, k] + C["c10"][a, k] * x + C["c20"][a, k] * x**2
        qv = C["c01"][a, k] * y + C["c02"][a, k] * y**2
        u = np.exp(qu - Mu[k])
        v = np.exp(qv - Mv[k] + sup[k])
        F[r0 : r0 + M] = (u[None, :] * L).astype(np.float32)
        G[r0 : r0 + M] = (
            v[None, :] * np.exp(C["c11"][a, k] * np.outer(nd, y))
        ).astype(np.float32)
        wrow[r0 : r0 + M] = C["w"][a, k]
        r0 += M
    return F, G, wrow, r0


def _host_inputs(params):
    import ml_dtypes

    C = _coefs(params)
    facs = [_factor_image(C, a) for a in range(A)]
    # per image-slot rank = max over cores (the NEFF fixes each slot's shape)
    Rs = tuple(
        min(max(16, (max(facs[IPC * c + im][3] for c in range(N_CORES)) + 15)
                // 16 * 16), RANK)
        for im in range(IPC)
    )
    in_maps = []
    for c in range(N_CORES):
        m = {}
        for im in range(IPC):
            F, G, wrow, _ = facs[IPC * c + im]
            fg = np.concatenate([F, wrow[:, None] * F, G], axis=1)[: Rs[im]]
            m[f"fg{im}"] = fg.astype(ml_dtypes.bfloat16)
        in_maps.append(m)
    return in_maps, Rs


# ----------------------------------------------------------------------------
# Bass kernel
# ----------------------------------------------------------------------------

_NC_CACHE = {}


def _build_nc(Rs):
    if Rs in _NC_CACHE:
        return _NC_CACHE[Rs]

    import concourse.bacc as bacc
    import concourse.mybir as mybir
    import concourse.tile as tile

    f32 = mybir.dt.float32
    bf16 = mybir.dt.bfloat16
    nc = bacc.Bacc("TRN2", target_bir_lowering=False, debug=False,
                   enable_asserts=False)

    fg_d = [nc.dram_tensor(f"fg{im}", (Rs[im], 768), bf16,
                           kind="ExternalInput").ap() for im in range(IPC)]
    # out[im, j_local, jt*256 + i]
    out_d = nc.dram_tensor("out", (IPC, 128, 512), bf16,
                           kind="ExternalOutput").ap()

    with tile.TileContext(nc) as tc:
        with ExitStack() as ctx:
            const_pool = ctx.enter_context(tc.tile_pool(name="const", bufs=1))
            ps_pool = ctx.enter_context(
                tc.tile_pool(name="ps", bufs=1, space="PSUM")
            )
            y_pool = ctx.enter_context(tc.tile_pool(name="y", bufs=1))

            # PE warm-up during the input DMA window (clock ramp: full rate
            # needs ~3us of continuous execution)
            warm_sb = const_pool.tile([128, 512], bf16, name="warm")
            nc.gpsimd.memset(warm_sb[:], 0.0)
            warm_ps = ps_pool.tile([128, 512], f32, name="warm_ps")
            for i in range(N_WARM):
                nc.tensor.matmul(warm_ps[:], warm_sb[:, 0:128], warm_sb[:],
                                 start=True, stop=True)

            # all inputs on the sync queue: the scalar engine's first body
            # instruction is the Reciprocal ACT table load, which would delay
            # any DMA issued from its queue
            fgs = [const_pool.tile([Rs[im], 768], bf16, name=f"fg{im}")
                   for im in range(IPC)]
            nc.sync.dma_start(fgs[0][:], fg_d[0][:])
            nc.sync.dma_start(fgs[1][:], fg_d[1][:])
            nc.sync.dma_start(fgs[2][:], fg_d[2][:])

            pss = []
            for im in range(IPC):
                # ps = [S1_jt0 | S1_jt1 | S2_jt0 | S2_jt1], 256 cols each
                ps = ps_pool.tile([128, 1024], f32, name=f"ps{im}")
                fg = fgs[im]
                for half, rhs in ((0, fg[:, 0:256]), (512, fg[:, 256:512])):
                    for jt in range(2):
                        nc.tensor.matmul(
                            ps[:, half + 256 * jt : half + 256 * (jt + 1)],
                            fg[:, 512 + 128 * jt : 512 + 128 * (jt + 1)],
                            rhs, start=True, stop=True,
                        )
                pss.append(ps)

            RECIP = mybir.ActivationFunctionType.Reciprocal
            for im in range(IPC):
                r = y_pool.tile([128, 512], f32, name=f"r{im}")
                y = y_pool.tile([128, 512], bf16, name=f"y{im}")
                # reciprocal on the otherwise-idle ACT engine; the DVE then
                # only multiplies. (ACT recip is less accurate than the DVE
                # custom op, but well inside this kernel's error budget.)
                nc.scalar.add_instruction(
                    mybir.InstActivation(
                        name=nc.get_next_instruction_name(),
                        func=RECIP,
                        ins=[
                            nc.scalar.lower_ap(pss[im][:, 0:512]),
                            mybir.ImmediateValue(dtype=f32, value=0.0),
                            mybir.ImmediateValue(dtype=f32, value=1.0),
                            mybir.ImmediateValue(dtype=f32, value=0.0),
                        ],
                        outs=[nc.scalar.lower_ap(r[:])],
                    )
                )
                nc.vector.tensor_mul(y[:], pss[im][:, 512:1024], r[:])
                if im < 2:
                    nc.sync.dma_start(out_d[im], y[:])
                else:
                    nc.sync.dma_start(out_d[im][:, 0:256], y[:, 0:256])
                    nc.scalar.dma_start(out_d[im][:, 256:512], y[:, 256:512])

    nc.compile()
    _NC_CACHE[Rs] = nc
    return nc


def _run(in_maps, Rs, **spmd_kwargs):
    from concourse.bass_utils import run_bass_kernel_spmd

    nc = _build_nc(Rs)
    return run_bass_kernel_spmd(
        nc, in_maps, core_ids=list(range(N_CORES)), **spmd_kwargs
    )


def _assemble(results):
    """results: 8 dicts with 'out' (IPC,2,128,256) -> (8,3,256,256)."""
    full = np.empty((A, H, W), dtype=np.float32)
    for c, res in enumerate(results):
        o = np.asarray(res["out"], dtype=np.float32)
        y = o.reshape(IPC, 128, 2, 256)
        img = y.transpose(0, 3, 2, 1).reshape(IPC, 256, 256)  # [im, i, j]
        full[IPC * c : IPC * (c + 1)] = img
    return full.reshape(8, 3, H, W)


def kernel(params, height, width):
    assert int(height) == H and int(width) == W
    in_maps, R = _host_inputs(params)
    res = _run(in_maps, R)
    return _assemble(res.results)


if __name__ == "__main__":
    params = np.random.RandomState(0).randn(8, 3, 7 * K).astype(np.float32)
    out = kernel(params, 256, 256)
    print("kernel ran, out", out.shape, out.dtype, np.isnan(out).sum())


# revision 37
# speedup vs baseline: 1.0841x; 1.0841x over previous
"""Trainium2 Bass kernel for the Gaussian-mixture image renderer (nn_MoE).

Math. out[a,h,w] = sum_k w_k e_k / sum_k e_k with
  e_k = exp(q), q = c00 + c10 x + c01 y + c20 x^2 + c11 x y + c02 y^2,
  x = lin[h], y = lin[w], lin = linspace(0,1,256), and c11 <= 0 always
  (c11 = -((l00+l10)^2 + l11^2)/2).
Factor e_k = u_k(x) * v_k(y) * exp(c11 x y) and Chebyshev-interpolate the
cross term in x with M_k nodes:
  exp(c x y) ~= sum_s L_s(x) exp(c x_s y)        (L_s = Lagrange basis)
so each image becomes a rank-R product (R = sum_k M_k <= 128):
  S1 = F^T G, S2 = (wF)^T G, out = S2/S1
  F[(k,s), i] = u_k(x_i) L_s(x_i),  G[(k,s), j] = v_k(y_j) exp(c11 x_s y_j).
M_k is looked up from a precomputed accuracy table (tol 3e-5); per-image
rank stays ~40-95 for this data. u,v are max-normalized so all factors
are <= O(1); the per-image scale cancels in S2/S1.

Device strategy (8 cores, data-parallel over images): core c renders
images 3c..3c+2. Per image: DMA F,G (128x256 f32); one DVE op builds
F2 = w*F; two f32r matmuls (lhsT = G j-halves, rhs = [F|F2]) produce
S1,S2 for 128 j-columns x 256 i; DVE reciprocal+multiply; DMA out.
~30 instructions/core total - no per-pixel exp, no q-build.
"""

import sys

if "/opt/trn_rl_repo" not in sys.path:
    sys.path.insert(0, "/opt/trn_rl_repo")

from contextlib import ExitStack

import numpy as np

K = 16
A = 24
H = W = 256
N_CORES = 8
IPC = 3  # images per core
RANK = 128
N_WARM = 5

# max |c11| handled by M interpolation nodes at tol 3e-5 (precomputed)
M_THRESH = [
    (1, 0.004),
    (2, 0.0217),
    (3, 0.1833),
    (4, 0.5740),
    (5, 1.2387),
    (6, 2.1383),
    (7, 3.3424),
    (8, 4.7308),
    (9, 6.3718),
    (10, 8.3716),
    (11, 10.4665),
    (12, 12.7648),
    (13, 15.1864),
    (14, 18.0673),
    (15, 19.9526),
]


# ----------------------------------------------------------------------------
# Host-side factorization
# ----------------------------------------------------------------------------

def _coefs(params):
    p = np.asarray(params, np.float64).reshape(A, 7 * K)
    mu0, mu1 = p[:, :K], p[:, K : 2 * K]
    wl = p[:, 2 * K : 3 * K]
    w = np.exp(wl - wl.max(1, keepdims=True))
    w /= w.sum(1, keepdims=True)
    raw = p[:, 3 * K : 7 * K].reshape(A, K, 2, 2)
    l00, l10, l11 = raw[:, :, 0, 0], raw[:, :, 1, 0], raw[:, :, 1, 1]
    s0 = l00 * l00 + l00 * l10
    s1 = l00 * l10 + l10 * l10 + l11 * l11
    s01 = s0 + s1
    return dict(
        c00=-0.5 * (s0 * mu0**2 + s01 * mu0 * mu1 + s1 * mu1**2),
        c10=0.5 * (2 * s0 * mu0 + s01 * mu1),
        c01=0.5 * (s01 * mu0 + 2 * s1 * mu1),
        c20=-0.5 * s0,
        c11=-0.5 * s01,
        c02=-0.5 * s1,
        w=w,
    )


def _qmax01(b, c):
    """max over t in [0,1] of b t + c t^2 (scalars)."""
    best = max(0.0, b + c)
    if abs(c) > 1e-30:
        t = -b / (2 * c)
        if 0.0 < t < 1.0:
            best = max(best, b * t + c * t * t)
    return best


def _pick_m(cabs):
    for m, thr in M_THRESH:
        if cabs <= thr:
            return m
    return min(15 + int(np.ceil((cabs - 20.0) / 2.0)), 24)


_L_CACHE = {}


def _nodes_lagrange(M):
    """Chebyshev nodes on [0,1] and Lagrange basis on the 256 grid."""
    if M in _L_CACHE:
        return _L_CACHE[M]
    x = np.linspace(0.0, 1.0, 256)
    if M == 1:
        nd = np.array([0.5])
        L = np.ones((1, 256))
    else:
        t = np.cos(np.pi * (2 * np.arange(M) + 1) / (2 * M))
        nd = 0.5 * (t + 1.0)
        wts = np.ones(M)
        for s in range(M):
            wts[s] = 1.0 / np.prod(nd[s] - np.delete(nd, s))
        d = x[None, :] - nd[:, None]
        exact = np.isclose(d, 0.0, atol=1e-13)
        d_safe = np.where(exact, 1.0, d)
        terms = wts[:, None] / d_safe
        L = terms / terms.sum(0)
        hit = exact.any(0)
        if hit.any():
            L[:, hit] = exact[:, hit].astype(float)
    _L_CACHE[M] = (nd, L)
    return nd, L


def _factor_image(C, a):
    """-> F (128,256) f32, G (128,256) f32, wrow (128,) f32."""
    x = np.linspace(0.0, 1.0, 256)
    y = x
    Mu = np.array(
        [C["c00"][a, k] + _qmax01(C["c10"][a, k], C["c20"][a, k]) for k in range(K)]
    )
    Mv = np.array([_qmax01(C["c01"][a, k], C["c02"][a, k]) for k in range(K)])
    sup = (Mu + Mv) - (Mu + Mv).max()
    Ms = [_pick_m(abs(C["c11"][a, k])) for k in range(K)]
    while sum(Ms) > RANK:
        Ms[int(np.argmax(Ms))] -= 1
    F = np.zeros((RANK, 256), np.float32)
    G = np.zeros((RANK, 256), np.float32)
    wrow = np.zeros(RANK, np.float32)
    r0 = 0
    for k in range(K):
        M = Ms[k]
        nd, L = _nodes_lagrange(M)
        qu = C["c00"][a, k] + C["c10"][a, k] * x + C["c20"][a, k] * x**2
        qv = C["c01"][a, k] * y + C["c02"][a, k] * y**2
        u = np.exp(qu - Mu[k])
        v = np.exp(qv - Mv[k] + sup[k])
        F[r0 : r0 + M] = (u[None, :] * L).astype(np.float32)
        G[r0 : r0 + M] = (
            v[None, :] * np.exp(C["c11"][a, k] * np.outer(nd, y))
        ).astype(np.float32)
        wrow[r0 : r0 + M] = C["w"][a, k]
        r0 += M
    return F, G, wrow, r0


def _host_inputs(params):
    import ml_dtypes

    C = _coefs(params)
    facs = [_factor_image(C, a) for a in range(A)]
    # per image-slot rank = max over cores (the NEFF fixes each slot's shape)
    Rs = tuple(
        min(max(16, (max(facs[IPC * c + im][3] for c in range(N_CORES)) + 15)
                // 16 * 16), RANK)
        for im in range(IPC)
    )
    in_maps = []
    for c in range(N_CORES):
        m = {}
        for im in range(IPC):
            F, G, wrow, _ = facs[IPC * c + im]
            fg = np.concatenate([F, wrow[:, None] * F, G], axis=1)[: Rs[im]]
            m[f"fg{im}"] = fg.astype(ml_dtypes.bfloat16)
        in_maps.append(m)
    return in_maps, Rs


# ----------------------------------------------------------------------------
# Bass kernel
# ----------------------------------------------------------------------------

_NC_CACHE = {}


def _build_nc(Rs):
    if Rs in _NC_CACHE:
        return _NC_CACHE[Rs]

    import concourse.bacc as bacc
    import concourse.mybir as mybir
    import concourse.tile as tile

    f32 = mybir.dt.float32
    bf16 = mybir.dt.bfloat16
    nc = bacc.Bacc("TRN2", target_bir_lowering=False, debug=False,
                   enable_asserts=False)

    fg_d = [nc.dram_tensor(f"fg{im}", (Rs[im], 768), bf16,
                           kind="ExternalInput").ap() for im in range(IPC)]
    # out[im, j_local, jt*256 + i]
    out_d = nc.dram_tensor("out", (IPC, 128, 512), bf16,
                           kind="ExternalOutput").ap()

    with tile.TileContext(nc) as tc:
        with ExitStack() as ctx:
            const_pool = ctx.enter_context(tc.tile_pool(name="const", bufs=1))
            ps_pool = ctx.enter_context(
                tc.tile_pool(name="ps", bufs=1, space="PSUM")
            )
            y_pool = ctx.enter_context(tc.tile_pool(name="y", bufs=1))

            # PE warm-up during the input DMA window (clock ramp: full rate
            # needs ~3us of continuous execution)
            warm_sb = const_pool.tile([128, 512], bf16, name="warm")
            nc.vector.memset(warm_sb[:], 0.0)
            warm_ps = ps_pool.tile([128, 512], f32, name="warm_ps")
            for i in range(N_WARM):
                nc.tensor.matmul(warm_ps[:], warm_sb[:, 0:128], warm_sb[:],
                                 start=True, stop=True)

            # all inputs on the sync queue: the scalar engine's first body
            # instruction is the Reciprocal ACT table load, which would delay
            # any DMA issued from its queue
            fgs = [const_pool.tile([Rs[im], 768], bf16, name=f"fg{im}")
                   for im in range(IPC)]
            nc.sync.dma_start(fgs[0][:], fg_d[0][:])
            nc.sync.dma_start(fgs[1][:], fg_d[1][:])
            nc.gpsimd.dma_start(fgs[2][:], fg_d[2][:])

            pss = []
            for im in range(IPC):
                # ps = [S1_jt0 | S1_jt1 | S2_jt0 | S2_jt1], 256 cols each
                ps = ps_pool.tile([128, 1024], f32, name=f"ps{im}")
                fg = fgs[im]
                for half, rhs in ((0, fg[:, 0:256]), (512, fg[:, 256:512])):
                    for jt in range(2):
                        nc.tensor.matmul(
                            ps[:, half + 256 * jt : half + 256 * (jt + 1)],
                            fg[:, 512 + 128 * jt : 512 + 128 * (jt + 1)],
                            rhs, start=True, stop=True,
                        )
                pss.append(ps)

            RECIP = mybir.ActivationFunctionType.Reciprocal
            for im in range(IPC):
                r = y_pool.tile([128, 512], f32, name=f"r{im}")
                y = y_pool.tile([128, 512], bf16, name=f"y{im}")
                # reciprocal on the otherwise-idle ACT engine; the DVE then
                # only multiplies. (ACT recip is less accurate than the DVE
                # custom op, but well inside this kernel's error budget.)
                nc.scalar.add_instruction(
                    mybir.InstActivation(
                        name=nc.get_next_instruction_name(),
                        func=RECIP,
                        ins=[
                            nc.scalar.lower_ap(pss[im][:, 0:512]),
                            mybir.ImmediateValue(dtype=f32, value=0.0),
                            mybir.ImmediateValue(dtype=f32, value=1.0),
                            mybir.ImmediateValue(dtype=f32, value=0.0),
                        ],
                        outs=[nc.scalar.lower_ap(r[:])],
                    )
                )
                nc.vector.tensor_mul(y[:], pss[im][:, 512:1024], r[:])
                if im < 2:
                    nc.sync.dma_start(out_d[im], y[:])
                else:
                    nc.sync.dma_start(out_d[im][:, 0:256], y[:, 0:256])
                    nc.scalar.dma_start(out_d[im][:, 256:512], y[:, 256:512])

    nc.compile()
    _NC_CACHE[Rs] = nc
    return nc


def _run(in_maps, Rs, **spmd_kwargs):
    from concourse.bass_utils import run_bass_kernel_spmd

    nc = _build_nc(Rs)
    return run_bass_kernel_spmd(
        nc, in_maps, core_ids=list(range(N_CORES)), **spmd_kwargs
    )


def _assemble(results):
    """results: 8 dicts with 'out' (IPC,2,128,256) -> (8,3,256,256)."""
    full = np.empty((A, H, W), dtype=np.float32)
    for c, res in enumerate(results):
        o = np.asarray(res["out"], dtype=np.float32)
        y = o.reshape(IPC, 128, 2, 256)
        img = y.transpose(0, 3, 2, 1).reshape(IPC, 256, 256)  # [im, i, j]
        full[IPC * c : IPC * (c + 1)] = img
    return full.reshape(8, 3, H, W)


def kernel(params, height, width):
    assert int(height) == H and int(width) == W
    in_maps, R = _host_inputs(params)
    res = _run(in_maps, R)
    return _assemble(res.results)


if __name__ == "__main__":
    params = np.random.RandomState(0).randn(8, 3, 7 * K).astype(np.float32)
    out = kernel(params, 256, 256)
    print("kernel ran, out", out.shape, out.dtype, np.isnan(out).sum())


# revision 38
# speedup vs baseline: 1.1630x; 1.0728x over previous
"""Trainium2 Bass kernel for the Gaussian-mixture image renderer (nn_MoE).

Math. out[a,h,w] = sum_k w_k e_k / sum_k e_k with
  e_k = exp(q), q = c00 + c10 x + c01 y + c20 x^2 + c11 x y + c02 y^2,
  x = lin[h], y = lin[w], lin = linspace(0,1,256), and c11 <= 0 always
  (c11 = -((l00+l10)^2 + l11^2)/2).
Factor e_k = u_k(x) * v_k(y) * exp(c11 x y) and Chebyshev-interpolate the
cross term in x with M_k nodes:
  exp(c x y) ~= sum_s L_s(x) exp(c x_s y)        (L_s = Lagrange basis)
so each image becomes a rank-R product (R = sum_k M_k <= 128):
  S1 = F^T G, S2 = (wF)^T G, out = S2/S1
  F[(k,s), i] = u_k(x_i) L_s(x_i),  G[(k,s), j] = v_k(y_j) exp(c11 x_s y_j).
M_k is looked up from a precomputed accuracy table (tol 3e-5); per-image
rank stays ~40-95 for this data. u,v are max-normalized so all factors
are <= O(1); the per-image scale cancels in S2/S1.

Device strategy (8 cores, data-parallel over images): core c renders
images 3c..3c+2. Per image: DMA F,G (128x256 f32); one DVE op builds
F2 = w*F; two f32r matmuls (lhsT = G j-halves, rhs = [F|F2]) produce
S1,S2 for 128 j-columns x 256 i; DVE reciprocal+multiply; DMA out.
~30 instructions/core total - no per-pixel exp, no q-build.
"""

import sys

if "/opt/trn_rl_repo" not in sys.path:
    sys.path.insert(0, "/opt/trn_rl_repo")

from contextlib import ExitStack

import numpy as np

K = 16
A = 24
H = W = 256
N_CORES = 8
IPC = 3  # images per core
RANK = 128
N_WARM = 5

# max |c11| handled by M interpolation nodes at tol 3e-5 (precomputed)
M_THRESH = [
    (1, 0.004),
    (2, 0.0217),
    (3, 0.1833),
    (4, 0.5740),
    (5, 1.2387),
    (6, 2.1383),
    (7, 3.3424),
    (8, 4.7308),
    (9, 6.3718),
    (10, 8.3716),
    (11, 10.4665),
    (12, 12.7648),
    (13, 15.1864),
    (14, 18.0673),
    (15, 19.9526),
]


# ----------------------------------------------------------------------------
# Host-side factorization
# ----------------------------------------------------------------------------

def _coefs(params):
    p = np.asarray(params, np.float64).reshape(A, 7 * K)
    mu0, mu1 = p[:, :K], p[:, K : 2 * K]
    wl = p[:, 2 * K : 3 * K]
    w = np.exp(wl - wl.max(1, keepdims=True))
    w /= w.sum(1, keepdims=True)
    raw = p[:, 3 * K : 7 * K].reshape(A, K, 2, 2)
    l00, l10, l11 = raw[:, :, 0, 0], raw[:, :, 1, 0], raw[:, :, 1, 1]
    s0 = l00 * l00 + l00 * l10
    s1 = l00 * l10 + l10 * l10 + l11 * l11
    s01 = s0 + s1
    return dict(
        c00=-0.5 * (s0 * mu0**2 + s01 * mu0 * mu1 + s1 * mu1**2),
        c10=0.5 * (2 * s0 * mu0 + s01 * mu1),
        c01=0.5 * (s01 * mu0 + 2 * s1 * mu1),
        c20=-0.5 * s0,
        c11=-0.5 * s01,
        c02=-0.5 * s1,
        w=w,
    )


def _qmax01(b, c):
    """max over t in [0,1] of b t + c t^2 (scalars)."""
    best = max(0.0, b + c)
    if abs(c) > 1e-30:
        t = -b / (2 * c)
        if 0.0 < t < 1.0:
            best = max(best, b * t + c * t * t)
    return best


def _pick_m(cabs):
    for m, thr in M_THRESH:
        if cabs <= thr:
            return m
    return min(15 + int(np.ceil((cabs - 20.0) / 2.0)), 24)


_L_CACHE = {}


def _nodes_lagrange(M):
    """Chebyshev nodes on [0,1] and Lagrange basis on the 256 grid."""
    if M in _L_CACHE:
        return _L_CACHE[M]
    x = np.linspace(0.0, 1.0, 256)
    if M == 1:
        nd = np.array([0.5])
        L = np.ones((1, 256))
    else:
        t = np.cos(np.pi * (2 * np.arange(M) + 1) / (2 * M))
        nd = 0.5 * (t + 1.0)
        wts = np.ones(M)
        for s in range(M):
            wts[s] = 1.0 / np.prod(nd[s] - np.delete(nd, s))
        d = x[None, :] - nd[:, None]
        exact = np.isclose(d, 0.0, atol=1e-13)
        d_safe = np.where(exact, 1.0, d)
        terms = wts[:, None] / d_safe
        L = terms / terms.sum(0)
        hit = exact.any(0)
        if hit.any():
            L[:, hit] = exact[:, hit].astype(float)
    _L_CACHE[M] = (nd, L)
    return nd, L


def _factor_image(C, a):
    """-> F (128,256) f32, G (128,256) f32, wrow (128,) f32."""
    x = np.linspace(0.0, 1.0, 256)
    y = x
    Mu = np.array(
        [C["c00"][a, k] + _qmax01(C["c10"][a, k], C["c20"][a, k]) for k in range(K)]
    )
    Mv = np.array([_qmax01(C["c01"][a, k], C["c02"][a, k]) for k in range(K)])
    sup = (Mu + Mv) - (Mu + Mv).max()
    Ms = [_pick_m(abs(C["c11"][a, k])) for k in range(K)]
    while sum(Ms) > RANK:
        Ms[int(np.argmax(Ms))] -= 1
    F = np.zeros((RANK, 256), np.float32)
    G = np.zeros((RANK, 256), np.float32)
    wrow = np.zeros(RANK, np.float32)
    r0 = 0
    for k in range(K):
        M = Ms[k]
        nd, L = _nodes_lagrange(M)
        qu = C["c00"][a, k] + C["c10"][a, k] * x + C["c20"][a, k] * x**2
        qv = C["c01"][a, k] * y + C["c02"][a, k] * y**2
        u = np.exp(qu - Mu[k])
        v = np.exp(qv - Mv[k] + sup[k])
        F[r0 : r0 + M] = (u[None, :] * L).astype(np.float32)
        G[r0 : r0 + M] = (
            v[None, :] * np.exp(C["c11"][a, k] * np.outer(nd, y))
        ).astype(np.float32)
        wrow[r0 : r0 + M] = C["w"][a, k]
        r0 += M
    return F, G, wrow, r0


def _host_inputs(params):
    import ml_dtypes

    C = _coefs(params)
    facs = [_factor_image(C, a) for a in range(A)]
    # per image-slot rank = max over cores (the NEFF fixes each slot's shape)
    Rs = tuple(
        min(max(16, (max(facs[IPC * c + im][3] for c in range(N_CORES)) + 15)
                // 16 * 16), RANK)
        for im in range(IPC)
    )
    in_maps = []
    for c in range(N_CORES):
        m = {}
        for im in range(IPC):
            F, G, wrow, _ = facs[IPC * c + im]
            fg = np.concatenate([F, wrow[:, None] * F, G], axis=1)[: Rs[im]]
            m[f"fg{im}"] = fg.astype(ml_dtypes.bfloat16)
        in_maps.append(m)
    return in_maps, Rs


# ----------------------------------------------------------------------------
# Bass kernel
# ----------------------------------------------------------------------------

_NC_CACHE = {}


def _build_nc(Rs):
    if Rs in _NC_CACHE:
        return _NC_CACHE[Rs]

    import concourse.bacc as bacc
    import concourse.mybir as mybir
    import concourse.tile as tile

    f32 = mybir.dt.float32
    bf16 = mybir.dt.bfloat16
    nc = bacc.Bacc("TRN2", target_bir_lowering=False, debug=False,
                   enable_asserts=False)

    fg_d = [nc.dram_tensor(f"fg{im}", (Rs[im], 768), bf16,
                           kind="ExternalInput").ap() for im in range(IPC)]
    # out[im, j_local, jt*256 + i]
    out_d = nc.dram_tensor("out", (IPC, 128, 512), bf16,
                           kind="ExternalOutput").ap()

    with tile.TileContext(nc) as tc:
        with ExitStack() as ctx:
            const_pool = ctx.enter_context(tc.tile_pool(name="const", bufs=1))
            ps_pool = ctx.enter_context(
                tc.tile_pool(name="ps", bufs=1, space="PSUM")
            )
            y_pool = ctx.enter_context(tc.tile_pool(name="y", bufs=1))

            # PE warm-up during the input DMA window (clock ramp: full rate
            # needs ~3us of continuous execution)
            warm_sb = const_pool.tile([128, 512], bf16, name="warm")
            nc.vector.memset(warm_sb[:], 0.0)
            warm_ps = ps_pool.tile([128, 512], f32, name="warm_ps")
            for i in range(N_WARM):
                nc.tensor.matmul(warm_ps[:], warm_sb[:, 0:128], warm_sb[:],
                                 start=True, stop=True)

            # all inputs on the sync queue: the scalar engine's first body
            # instruction is the Reciprocal ACT table load, which would delay
            # any DMA issued from its queue
            fgs = [const_pool.tile([Rs[im], 768], bf16, name=f"fg{im}")
                   for im in range(IPC)]
            nc.sync.dma_start(fgs[0][:], fg_d[0][:])
            nc.sync.dma_start(fgs[1][:], fg_d[1][:])
            nc.gpsimd.dma_start(fgs[2][:, 0:384], fg_d[2][:, 0:384])
            nc.sync.dma_start(fgs[2][:, 384:768], fg_d[2][:, 384:768])

            pss = []
            for im in range(IPC):
                # separate S1/S2 PSUM tiles: the reciprocal then only waits
                # on the two S1 matmuls, not all four
                psa = ps_pool.tile([128, 512], f32, name=f"psa{im}")
                psb = ps_pool.tile([128, 512], f32, name=f"psb{im}")
                fg = fgs[im]
                for ps, rhs in ((psa, fg[:, 0:256]), (psb, fg[:, 256:512])):
                    for jt in range(2):
                        nc.tensor.matmul(
                            ps[:, 256 * jt : 256 * (jt + 1)],
                            fg[:, 512 + 128 * jt : 512 + 128 * (jt + 1)],
                            rhs, start=True, stop=True,
                        )
                pss.append((psa, psb))

            RECIP = mybir.ActivationFunctionType.Reciprocal
            for im in range(IPC):
                r = y_pool.tile([128, 512], f32, name=f"r{im}")
                y = y_pool.tile([128, 512], bf16, name=f"y{im}")
                # reciprocal on the otherwise-idle ACT engine; the DVE then
                # only multiplies. (ACT recip is less accurate than the DVE
                # custom op, but well inside this kernel's error budget.)
                nc.scalar.add_instruction(
                    mybir.InstActivation(
                        name=nc.get_next_instruction_name(),
                        func=RECIP,
                        ins=[
                            nc.scalar.lower_ap(pss[im][0][:]),
                            mybir.ImmediateValue(dtype=f32, value=0.0),
                            mybir.ImmediateValue(dtype=f32, value=1.0),
                            mybir.ImmediateValue(dtype=f32, value=0.0),
                        ],
                        outs=[nc.scalar.lower_ap(r[:])],
                    )
                )
                nc.vector.tensor_mul(y[:], pss[im][1][:], r[:])
                if im < 2:
                    nc.sync.dma_start(out_d[im], y[:])
                else:
                    nc.sync.dma_start(out_d[im][:, 0:256], y[:, 0:256])
                    nc.scalar.dma_start(out_d[im][:, 256:512], y[:, 256:512])

    nc.compile()
    _NC_CACHE[Rs] = nc
    return nc


def _run(in_maps, Rs, **spmd_kwargs):
    from concourse.bass_utils import run_bass_kernel_spmd

    nc = _build_nc(Rs)
    return run_bass_kernel_spmd(
        nc, in_maps, core_ids=list(range(N_CORES)), **spmd_kwargs
    )


def _assemble(results):
    """results: 8 dicts with 'out' (IPC,2,128,256) -> (8,3,256,256)."""
    full = np.empty((A, H, W), dtype=np.float32)
    for c, res in enumerate(results):
        o = np.asarray(res["out"], dtype=np.float32)
        y = o.reshape(IPC, 128, 2, 256)
        img = y.transpose(0, 3, 2, 1).reshape(IPC, 256, 256)  # [im, i, j]
        full[IPC * c : IPC * (c + 1)] = img
    return full.reshape(8, 3, H, W)


def kernel(params, height, width):
    assert int(height) == H and int(width) == W
    in_maps, R = _host_inputs(params)
    res = _run(in_maps, R)
    return _assemble(res.results)


if __name__ == "__main__":
    params = np.random.RandomState(0).randn(8, 3, 7 * K).astype(np.float32)
    out = kernel(params, 256, 256)
    print("kernel ran, out", out.shape, out.dtype, np.isnan(out).sum())
